# revision 42
# baseline (speedup 1.0000x reference)
"""Masked dot-product attention (B=2,H=16,L=2048,D=128) on 8 trn2 NeuronCores.

Strategy (v7 — clean-queue, warm-start evolution of v6):
  - Shard batch*heads: core c handles (b=0,h=2c),(0,2c+1),(1,2c),(1,2c+1) -> 4 slots.
  - Host ships Q and K transposed to [d, seq] bf16, V natural [seq, d] bf16;
    K/V columns at positions >= valid_len are zeroed so masking costs nothing
    on device (exp(0)=1 contributes a host-subtracted constant to the
    denominator and nothing to O^T).
  - Per key tile j and 512-wide q block:
      S^T[k,q] = kT_j^T qT   (PE, bf16)
      pT = exp(S^T/sqrt(D))  (ACT, fused scale, bf16 out)
      O^T[d,q] += v_j^T pT   (PE, bf16, PSUM accum)
    plus DVE accumulation of the per-partition softmax denominator partials
    (host does the final 128-fold) and DVE eviction of O^T.
  - v7 changes vs v6 (trace-driven; 57.4us -> 50.5us measured):
      * The scalar (Activation) queue runs ONLY the ~27 exp ACTIVATEs plus
        the three tiny critical head loads and the final lacc stores --
        every other DMA trigger (~600ns each) and its sem glue moved off
        it, since exp (~34us busy) is the pacing engine.
      * Critical head loads (q block0, k tile0, v tile0) ride the scalar
        HWDGE queue whose preamble ends ~0.8us before sync's; the rest of
        the loads ride sync ordered by first use (k tiles 1.., q block1,
        v tiles 1..). First-DMA ring latency ~2.3us is the head floor.
      * A GLOBAL software pipeline across slot boundaries: the (slot,
        block, tile-group) units are flattened into one list with the
        S-matmul phase running two units ahead, so the PE never drains at
        slot switches (each boundary stall was ~1us).
      * 5 PE warmup matmuls + 1 exp on zeroed scratch during the DMA head:
        the ACT table load (~1.3us) and the HAM clock-gate ramp (~3.4us)
        happen before real work needs them. (Without this the PE runs at
        1.2GHz for the first ~12us.)
      * Mid-kernel stores ride the GpSimd SWDGE queue; the last slot's
        stores split between sync (oT per block) and scalar (lacc), and
        its final tile-group has n=1 so the last exp->PV->evict->store
        chain is minimal.
      * Slots run smallest-K first and smallest-K last (minimal head and
        tail), the two big slots in the middle.
  - Known fixed overheads in the measured window: ~7us of per-semaphore
    reset postamble (255 sems, Tensor engine serial at ~130ns each) plus
    ~1.4us final barrier and ~2us store drain; these are framework-fixed.
  - Beware: the device flips between PE 2.4GHz (warm MM dur ~379ns) and a
    P0 power-throttled 2.0GHz (~454ns) depending on recent load; the same
    kernel measures ~50.5us vs ~59us. Compare runs only within the same
    clock class.
"""

import math

import numpy as np

try:
    import concourse.bass as bass
except ImportError:  # pragma: no cover
    import sys

    sys.path.append("/opt/trn_rl_repo")
    import concourse.bass as bass

import concourse.mybir as mybir
import concourse.tile as tile
from concourse import bacc
from concourse.bass_utils import run_bass_kernel_spmd

B, H, L, D = 2, 16, 2048, 128
NCORES = 8
HPC = H // NCORES  # heads per core per batch
SLOTS = B * HPC  # bh slots per core
INV_SQRT_D = 1.0 / math.sqrt(D)
F32 = mybir.dt.float32
BF16 = mybir.dt.bfloat16
QB = 4  # q blocks
QBW = L // QB  # 512 q per block
EXPF = mybir.ActivationFunctionType.Exp

_cache: dict = {}


def _build(K0: int, K1: int):
    """Build+compile the per-core program for K0/K1 valid key tiles."""
    Ks = [K0, K0, K1, K1]
    KM = max(K0, K1)
    nc = bacc.Bacc("TRN2", target_bir_lowering=False, debug=False, num_devices=NCORES)
    qT = nc.dram_tensor("qT", [SLOTS, D, L], BF16, kind="ExternalInput")
    kT = nc.dram_tensor("kT", [SLOTS, D, KM * 128], BF16, kind="ExternalInput")
    v = nc.dram_tensor("v", [SLOTS, KM * 128, D], BF16, kind="ExternalInput")
    oT = nc.dram_tensor("oT", [SLOTS, D, L], BF16, kind="ExternalOutput")
    lout = nc.dram_tensor("lout", [SLOTS, 128, QB, QBW], BF16, kind="ExternalOutput")

    with tile.TileContext(nc) as tc:
        with (
            tc.tile_pool(name="warm", bufs=1) as warmp,
            tc.tile_pool(name="io", bufs=2) as iop,
            tc.tile_pool(name="qp", bufs=2) as qp,
            tc.tile_pool(name="work", bufs=6) as workp,
            tc.tile_pool(name="lp", bufs=3) as lp,
            tc.tile_pool(name="op", bufs=3) as op_,
            tc.tile_pool(name="psst", bufs=2, space="PSUM") as psst,
            tc.tile_pool(name="psot", bufs=2, space="PSUM") as psot,
        ):
            # slot order: biggest K first (denser early PE phases flip the
            # HAM clock-gate sooner), smallest last (short tail).
            ss = sorted(range(SLOTS), key=lambda x: Ks[x])
            order = [ss[3], ss[2], ss[0], ss[1]]

            # ---- critical first loads ride the scalar queue, whose preamble
            # finishes ~0.8us before sync's; q block0 (the 128KB long pole)
            # first. They precede the ACT table load in the scalar FIFO.
            sA = order[0]
            KvA = Ks[sA]
            # (measured dead end: tiny "ring warmup" dummy loads ahead of
            # the critical loads just delay them — the ~2.3us first-DMA
            # latency is per-DMA pipeline depth, not one-time queue state)
            qsA = qp.tile([128, QB, QBW], BF16, tag="qs")
            nc.scalar.dma_start(out=qsA[:, 0, :], in_=qT[sA, :, :QBW])
            ktsA = iop.tile([128, KM, 128], BF16, tag="kts")
            nc.scalar.dma_start(
                out=ktsA[:, 0:1, :],
                in_=kT[sA, :, 0:128].rearrange("d (t p) -> d t p", p=128),
            )
            vnA = iop.tile([128, KM, 128], BF16, tag="vn")
            nc.scalar.dma_start(
                out=vnA[:, 0:1, :],
                in_=v[sA, :128, :].rearrange("(t p) d -> p t d", p=128),
            )
            # ---- warmup: prime the ACT exp table (~1.3us load) and the PE
            # HAM clock gate during the DMA head. The 6 matmuls (~2.6us cold)
            # end about when the first real data lands, so real matmuls run
            # at 2.4GHz from the start instead of ~12us of half-clock.
            wsb = warmp.tile([128, 512], BF16, tag="wsb")
            nc.gpsimd.memset(wsb[:, :], 0.0)
            wact = warmp.tile([128, 64], BF16, tag="wact")
            nc.scalar.activation(wact[:, :], wsb[:, :64], EXPF, scale=INV_SQRT_D)
            wst = psst.tile([128, 3, 512], F32, tag="st")
            for _ in range(5):
                nc.tensor.matmul(
                    wst[:, 0, :], wsb[:, :128], wsb[:, :], start=True, stop=True
                )

            def emit_load_k(s):
                Kv = Ks[s]
                kts = iop.tile([128, KM, 128], BF16, tag="kts")
                nc.sync.dma_start(
                    out=kts[:, :Kv, :],
                    in_=kT[s, :, : Kv * 128].rearrange("d (t p) -> d t p", p=128),
                )
                return kts

            def emit_load_v(s):
                Kv = Ks[s]
                vn = iop.tile([128, KM, 128], BF16, tag="vn")
                nc.sync.dma_start(
                    out=vn[:, :Kv, :],
                    in_=v[s, : Kv * 128, :].rearrange("(t p) d -> p t d", p=128),
                )
                return vn

            def emit_load_q2(s):
                # q in two halves so the first 2 blocks land early
                qs = qp.tile([128, QB, QBW], BF16, tag="qs")
                for h in range(2):
                    nc.sync.dma_start(
                        out=qs[:, 2 * h : 2 * h + 2, :],
                        in_=qT[s, :, h * L // 2 : (h + 1) * L // 2].rearrange(
                            "d (b w) -> d b w", b=2
                        ),
                    )
                return qs

            # ---- rest of the load schedule on the sync queue, in order of
            # first use: k tiles 1-2 (2nd exp unit), q block1 (3rd unit),
            # v tiles 1-2 (2nd unit's PV), then the tail k/v tiles.
            kA1 = min(KvA, 3)
            if KvA > 1:
                nc.sync.dma_start(
                    out=ktsA[:, 1:kA1, :],
                    in_=kT[sA, :, 128 : kA1 * 128].rearrange(
                        "d (t p) -> d t p", p=128
                    ),
                )
            nc.sync.dma_start(out=qsA[:, 1, :], in_=qT[sA, :, QBW : 2 * QBW])
            if KvA > 1:
                nc.sync.dma_start(
                    out=vnA[:, 1:kA1, :],
                    in_=v[sA, 128 : kA1 * 128, :].rearrange(
                        "(t p) d -> p t d", p=128
                    ),
                )
            if KvA > kA1:
                nc.sync.dma_start(
                    out=ktsA[:, kA1:KvA, :],
                    in_=kT[sA, :, kA1 * 128 : KvA * 128].rearrange(
                        "d (t p) -> d t p", p=128
                    ),
                )
                nc.sync.dma_start(
                    out=vnA[:, kA1:KvA, :],
                    in_=v[sA, kA1 * 128 : KvA * 128, :].rearrange(
                        "(t p) d -> p t d", p=128
                    ),
                )
            # slot B's k/v next (needed ~6us in), then A's q tail, then B's q
            sB = order[1]
            ktsB = emit_load_k(sB)
            vnB = emit_load_v(sB)
            nc.sync.dma_start(
                out=qsA[:, 2:4, :],
                in_=qT[sA, :, 2 * QBW :].rearrange("d (b w) -> d b w", b=2),
            )
            qsB = emit_load_q2(sB)

            preload = {sA: (ktsA, vnA, qsA), sB: (ktsB, vnB, qsB)}

            # ---- build the GLOBAL unit list: (slot-idx, qb, j0, n, flags).
            # Flattening across slots lets the S-matmul lookahead span slot
            # boundaries, so the PE never drains at a slot switch.
            # unit = (idx, qb, j0, n, is_first, is_last, c0, cw): key tiles
            # [j0, j0+n) x q columns [c0, c0+cw) of block qb of slot idx.
            def slot_units(idx):
                s = order[idx]
                Kv = Ks[s]
                units = []
                for qb in range(QB):
                    if idx == 0 and qb == 0:
                        # fill phase: (1, rest) tile groups so the first exp
                        # fires as soon as k tile 0 + q block 0 land
                        sub = [(0, 1, 0, QBW, True)]
                        if Kv >= 3:
                            sub.append((1, 2, 0, QBW, False))
                        elif Kv == 2:
                            sub.append((1, 1, 0, QBW, False))
                        j = sum(x[1] for x in sub)
                        while j < Kv:
                            n = min(3, Kv - j)
                            sub.append((j, n, 0, QBW, False))
                            j += n
                        n_u = len(sub)
                        for gi, (j0, n, c0, cw, lf) in enumerate(sub):
                            units.append(
                                (idx, qb, j0, n, gi == 0, gi == n_u - 1, c0, cw, lf)
                            )
                        continue
                    groups = []
                    j = 0
                    if idx == SLOTS - 1 and qb == QB - 1 and Kv == 3:
                        # last block of the last slot ends with a 1-tile
                        # group so the final exp->PV->evict->store chain is
                        # as short as possible
                        groups = [(0, 2), (2, 1)]
                    else:
                        while j < Kv:
                            n = min(3, Kv - j)
                            groups.append((j, n))
                            j += n
                    for gi, (j0, n) in enumerate(groups):
                        units.append(
                            (
                                idx,
                                qb,
                                j0,
                                n,
                                gi == 0,
                                gi == len(groups) - 1,
                                0,
                                QBW,
                                gi == 0,
                            )
                        )
                return units

            gunits = []
            slot_first_u = []
            for idx in range(SLOTS):
                slot_first_u.append(len(gunits))
                gunits += slot_units(idx)
            U = len(gunits)
            sts = [None] * U
            state = {}  # per-slot tiles, filled lazily

            def slot_state(idx):
                if idx not in state:
                    s = order[idx]
                    kts, vn, qs = preload.pop(s)
                    state[idx] = {
                        "kts": kts,
                        "vn": vn,
                        "qs": qs,
                        "laccs": lp.tile(
                            [128, QB, QBW], BF16, tag="laccs", name="laccs"
                        ),
                        "o_sb": op_.tile(
                            [128, QB, QBW], BF16, tag="o_sb", name="o_sb"
                        ),
                        "oT_pss": [None] * QB,
                    }
                return state[idx]

            def emit_s(u):
                idx, qb, j0, n, _, _, c0, cw, _ = gunits[u]
                st_ = slot_state(idx)
                # prefetch the next slot's tensors when a slot's S-phase opens
                if idx + 1 < SLOTS and u == slot_first_u[idx]:
                    nxt = order[idx + 1]
                    if nxt in preload or idx + 1 in state:
                        pass
                    else:
                        preload[nxt] = (
                            emit_load_k(nxt),
                            emit_load_v(nxt),
                            emit_load_q2(nxt),
                        )
                st = psst.tile([128, 3, 512], F32, tag="st")
                for jj in range(n):
                    if cw == QBW:
                        nc.tensor.matmul(
                            st[:, jj, :],
                            st_["kts"][:, j0 + jj, :],
                            st_["qs"][:, qb, :],
                            start=True,
                            stop=True,
                        )
                    else:
                        nc.tensor.matmul(
                            st[:, jj, :cw],
                            st_["kts"][:, j0 + jj, :],
                            st_["qs"][:, qb, c0 : c0 + cw],
                            start=True,
                            stop=True,
                        )
                sts[u] = st

            def emit_consume(u):
                idx, qb, j0, n, is_first, is_last, c0, cw, lacc_first = gunits[u]
                s = order[idx]
                Kv = Ks[s]
                last_slot = idx == SLOTS - 1
                st_ = state[idx]
                laccs, o_sb = st_["laccs"], st_["o_sb"]
                st = sts[u]
                full = cw == QBW
                lacc = laccs[:, qb, :] if full else laccs[:, qb, c0 : c0 + cw]
                if is_first:
                    st_["oT_pss"][qb] = psot.tile(
                        [128, QBW], F32, tag="oT", name="oT_ps"
                    )
                oT_ps = st_["oT_pss"][qb]
                pT = workp.tile([128, 3, QBW], BF16, tag="pT")
                if full:
                    nc.scalar.activation(
                        pT[:, :n, :], st[:, :n, :], EXPF, scale=INV_SQRT_D
                    )
                else:
                    nc.scalar.activation(
                        pT[:, :n, :cw], st[:, :n, :cw], EXPF, scale=INV_SQRT_D
                    )
                for jj in range(n):
                    jf = j0 + jj
                    nc.tensor.matmul(
                        oT_ps if full else oT_ps[:, c0 : c0 + cw],
                        st_["vn"][:, jf, :],
                        pT[:, jj, :] if full else pT[:, jj, :cw],
                        start=(jf == 0),
                        stop=(jf == Kv - 1),
                    )
                # denominator accumulation on the DVE (bf16 2x rate)
                base = 0
                if lacc_first:
                    if n >= 2:
                        nc.vector.tensor_add(
                            lacc,
                            pT[:, 0, :] if full else pT[:, 0, :cw],
                            pT[:, 1, :] if full else pT[:, 1, :cw],
                        )
                        base = 2
                    else:
                        nc.vector.tensor_copy(
                            lacc, pT[:, 0, :] if full else pT[:, 0, :cw]
                        )
                        base = 1
                for jj in range(base, n):
                    nc.vector.tensor_add(
                        lacc, lacc, pT[:, jj, :] if full else pT[:, jj, :cw]
                    )
                if is_last:
                    # evict O^T (cast to bf16) on the DVE
                    nc.vector.tensor_copy(o_sb[:, qb, :], oT_ps)
                    if last_slot:
                        # per-block stores; oT rides sync, the lacc blocks
                        # of the second half ride the (idle by now) scalar
                        # queue so the end-of-kernel drain uses two rings
                        # in parallel, ~128KB each
                        nc.sync.dma_start(
                            out=oT[s, :, qb * QBW : (qb + 1) * QBW],
                            in_=o_sb[:, qb, :],
                        )
                        if qb == 1:
                            nc.sync.dma_start(
                                out=lout[s, :, 0:2, :],
                                in_=laccs[:, 0:2, :],
                            )
                        elif qb >= 2:
                            nc.scalar.dma_start(
                                out=lout[s, :, qb : qb + 1, :],
                                in_=laccs[:, qb : qb + 1, :],
                            )
                    else:
                        # mid-kernel stores ride SWDGE (gpsimd), keeping
                        # both HWDGE queues free for loads
                        if qb % 2 == 1:
                            nc.gpsimd.dma_start(
                                out=oT[s, :, (qb - 1) * QBW : (qb + 1) * QBW]
                                .rearrange("d (b w) -> d b w", b=2),
                                in_=o_sb[:, qb - 1 : qb + 1, :],
                            )
                        if qb == QB - 1:
                            nc.gpsimd.dma_start(
                                out=lout[s, :, :, :], in_=laccs[:, :, :]
                            )

            for u in range(min(2, U)):
                emit_s(u)
            for u in range(2, U):
                emit_s(u)
                emit_consume(u - 2)
            for u in range(max(0, U - 2), U):
                emit_consume(u)
    nc.compile()
    return nc


def _get_program(K0: int, K1: int):
    key = (K0, K1)
    if key not in _cache:
        _cache[key] = _build(K0, K1)
    return _cache[key]


def _run(q, k, v, valid_lens, trace=False):
    import ml_dtypes

    BF = ml_dtypes.bfloat16
    q = np.asarray(q, dtype=np.float32)
    k = np.asarray(k, dtype=np.float32)
    v = np.asarray(v, dtype=np.float32)
    vl = np.asarray(valid_lens).astype(np.int64)
    K0 = int(max(1, -(-vl[0] // 128)))
    K1 = int(max(1, -(-vl[1] // 128)))
    KM = max(K0, K1)
    nc = _get_program(K0, K1)

    Ks = [K0, K0, K1, K1]
    bs = [0, 0, 1, 1]
    nmask = [Ks[i] * 128 - int(vl[bs[i]]) for i in range(SLOTS)]

    # zero masked key positions once for the whole tensor (shared across cores)
    kz = k[:, :, : KM * 128, :].copy()
    vz = v[:, :, : KM * 128, :].astype(BF)
    for b in range(B):
        kz[b, :, vl[b] :, :] = 0.0
        vz[b, :, vl[b] :, :] = 0.0
    # [B, H, D, KM*128] transposed keys / queries in bf16
    kzT = np.ascontiguousarray(kz.transpose(0, 1, 3, 2)).astype(BF)
    qT_full = np.ascontiguousarray(q.transpose(0, 1, 3, 2)).astype(BF)

    in_maps = []
    for c in range(NCORES):
        h0, h1 = 2 * c, 2 * c + 1
        qts = np.ascontiguousarray(
            np.stack([qT_full[0, h0], qT_full[0, h1], qT_full[1, h0], qT_full[1, h1]])
        )
        kts = np.ascontiguousarray(
            np.stack([kzT[0, h0], kzT[0, h1], kzT[1, h0], kzT[1, h1]])
        )
        vs = np.ascontiguousarray(
            np.stack([vz[0, h0], vz[0, h1], vz[1, h0], vz[1, h1]])
        )
        in_maps.append({"qT": qts, "kT": kts, "v": vs})

    try:
        res = run_bass_kernel_spmd(
            nc, in_maps, core_ids=list(range(NCORES)), trace=trace
        )
    except Exception:
        # transient device wedges (NRT_EXEC_UNIT_UNRECOVERABLE) have been
        # observed to clear on retry
        res = run_bass_kernel_spmd(
            nc, in_maps, core_ids=list(range(NCORES)), trace=trace
        )

    outp = np.empty((B, H, L, D), dtype=np.float32)
    for c in range(NCORES):
        oT_dev = res.results[c]["oT"]
        l_dev = res.results[c]["lout"]
        h0, h1 = 2 * c, 2 * c + 1
        for i, (b, h) in enumerate([(0, h0), (0, h1), (1, h0), (1, h1)]):
            l = l_dev[i].astype(np.float32).sum(axis=0).reshape(L) - nmask[i]
            outp[b, h] = oT_dev[i].astype(np.float32).T / l[:, None]
    return outp, res


def kernel(q, k, v, valid_lens):
    outp, _ = _run(q, k, v, valid_lens, trace=False)
    return outp


# revision 43
# speedup vs baseline: 1.0172x; 1.0172x over previous
"""Masked dot-product attention (B=2,H=16,L=2048,D=128) on 8 trn2 NeuronCores.

Strategy (v7 — clean-queue, warm-start evolution of v6):
  - Shard batch*heads: core c handles (b=0,h=2c),(0,2c+1),(1,2c),(1,2c+1) -> 4 slots.
  - Host ships Q and K transposed to [d, seq] bf16, V natural [seq, d] bf16;
    K/V columns at positions >= valid_len are zeroed so masking costs nothing
    on device (exp(0)=1 contributes a host-subtracted constant to the
    denominator and nothing to O^T).
  - Per key tile j and 512-wide q block:
      S^T[k,q] = kT_j^T qT   (PE, bf16)
      pT = exp(S^T/sqrt(D))  (ACT, fused scale, bf16 out)
      O^T[d,q] += v_j^T pT   (PE, bf16, PSUM accum)
    plus DVE accumulation of the per-partition softmax denominator partials
    (host does the final 128-fold) and DVE eviction of O^T.
  - v7 changes vs v6 (trace-driven; 57.4us -> 50.5us measured):
      * The scalar (Activation) queue runs ONLY the ~27 exp ACTIVATEs plus
        the three tiny critical head loads and the final lacc stores --
        every other DMA trigger (~600ns each) and its sem glue moved off
        it, since exp (~34us busy) is the pacing engine.
      * Critical head loads (q block0, k tile0, v tile0) ride the scalar
        HWDGE queue whose preamble ends ~0.8us before sync's; the rest of
        the loads ride sync ordered by first use (k tiles 1.., q block1,
        v tiles 1..). First-DMA ring latency ~2.3us is the head floor.
      * A GLOBAL software pipeline across slot boundaries: the (slot,
        block, tile-group) units are flattened into one list with the
        S-matmul phase running two units ahead, so the PE never drains at
        slot switches (each boundary stall was ~1us).
      * 5 PE warmup matmuls + 1 exp on zeroed scratch during the DMA head:
        the ACT table load (~1.3us) and the HAM clock-gate ramp (~3.4us)
        happen before real work needs them. (Without this the PE runs at
        1.2GHz for the first ~12us.)
      * Mid-kernel stores ride the GpSimd SWDGE queue; the last slot's
        stores split between sync (oT per block) and scalar (lacc), and
        its final tile-group has n=1 so the last exp->PV->evict->store
        chain is minimal.
      * Slots run smallest-K first and smallest-K last (minimal head and
        tail), the two big slots in the middle.
  - Known fixed overheads in the measured window: ~7us of per-semaphore
    reset postamble (255 sems, Tensor engine serial at ~130ns each) plus
    ~1.4us final barrier and ~2us store drain; these are framework-fixed.
  - Beware: the device flips between PE 2.4GHz (warm MM dur ~379ns) and a
    P0 power-throttled 2.0GHz (~454ns) depending on recent load; the same
    kernel measures ~50.5us vs ~59us. Compare runs only within the same
    clock class.
"""

import math

import numpy as np

try:
    import concourse.bass as bass
except ImportError:  # pragma: no cover
    import sys

    sys.path.append("/opt/trn_rl_repo")
    import concourse.bass as bass

import concourse.mybir as mybir
import concourse.tile as tile
from concourse import bacc
from concourse.bass_utils import run_bass_kernel_spmd

B, H, L, D = 2, 16, 2048, 128
NCORES = 8
HPC = H // NCORES  # heads per core per batch
SLOTS = B * HPC  # bh slots per core
INV_SQRT_D = 1.0 / math.sqrt(D)
F32 = mybir.dt.float32
BF16 = mybir.dt.bfloat16
QB = 4  # q blocks
QBW = L // QB  # 512 q per block
EXPF = mybir.ActivationFunctionType.Exp

_cache: dict = {}


def _build(K0: int, K1: int):
    """Build+compile the per-core program for K0/K1 valid key tiles."""
    Ks = [K0, K0, K1, K1]
    KM = max(K0, K1)
    nc = bacc.Bacc("TRN2", target_bir_lowering=False, debug=False, num_devices=NCORES)
    qT = nc.dram_tensor("qT", [SLOTS, D, L], BF16, kind="ExternalInput")
    kT = nc.dram_tensor("kT", [SLOTS, D, KM * 128], BF16, kind="ExternalInput")
    v = nc.dram_tensor("v", [SLOTS, KM * 128, D], BF16, kind="ExternalInput")
    oT = nc.dram_tensor("oT", [SLOTS, D, L], BF16, kind="ExternalOutput")
    lout = nc.dram_tensor("lout", [SLOTS, 128, QB, QBW], BF16, kind="ExternalOutput")

    with tile.TileContext(nc) as tc:
        with (
            tc.tile_pool(name="warm", bufs=1) as warmp,
            tc.tile_pool(name="io", bufs=2) as iop,
            tc.tile_pool(name="qp", bufs=2) as qp,
            tc.tile_pool(name="work", bufs=6) as workp,
            tc.tile_pool(name="lp", bufs=3) as lp,
            tc.tile_pool(name="op", bufs=3) as op_,
            tc.tile_pool(name="psst", bufs=2, space="PSUM") as psst,
            tc.tile_pool(name="psot", bufs=2, space="PSUM") as psot,
        ):
            # slot order: biggest K first (denser early PE phases flip the
            # HAM clock-gate sooner), smallest last (short tail).
            ss = sorted(range(SLOTS), key=lambda x: Ks[x])
            order = [ss[3], ss[2], ss[0], ss[1]]

            # ---- critical first loads ride the scalar queue, whose preamble
            # finishes ~0.8us before sync's; q block0 (the 128KB long pole)
            # first. They precede the ACT table load in the scalar FIFO.
            sA = order[0]
            KvA = Ks[sA]
            # (measured dead end: tiny "ring warmup" dummy loads ahead of
            # the critical loads just delay them — the ~2.3us first-DMA
            # latency is per-DMA pipeline depth, not one-time queue state)
            qsA = qp.tile([128, QB, QBW], BF16, tag="qs")
            nc.scalar.dma_start(out=qsA[:, 0, :], in_=qT[sA, :, :QBW])
            ktsA = iop.tile([128, KM, 128], BF16, tag="kts")
            nc.scalar.dma_start(
                out=ktsA[:, 0:1, :],
                in_=kT[sA, :, 0:128].rearrange("d (t p) -> d t p", p=128),
            )
            vnA = iop.tile([128, KM, 128], BF16, tag="vn")
            nc.scalar.dma_start(
                out=vnA[:, 0:1, :],
                in_=v[sA, :128, :].rearrange("(t p) d -> p t d", p=128),
            )
            # ---- warmup: prime the ACT exp table (~1.3us load) and the PE
            # HAM clock gate during the DMA head. The 6 matmuls (~2.6us cold)
            # end about when the first real data lands, so real matmuls run
            # at 2.4GHz from the start instead of ~12us of half-clock.
            wsb = warmp.tile([128, 512], BF16, tag="wsb")
            nc.gpsimd.memset(wsb[:, :], 0.0)
            wact = warmp.tile([128, 64], BF16, tag="wact")
            nc.scalar.activation(wact[:, :], wsb[:, :64], EXPF, scale=INV_SQRT_D)
            wst = psst.tile([128, 3, 512], F32, tag="st")
            for _ in range(5):
                nc.tensor.matmul(
                    wst[:, 0, :], wsb[:, :128], wsb[:, :], start=True, stop=True
                )

            def emit_load_k(s):
                Kv = Ks[s]
                kts = iop.tile([128, KM, 128], BF16, tag="kts")
                nc.sync.dma_start(
                    out=kts[:, :Kv, :],
                    in_=kT[s, :, : Kv * 128].rearrange("d (t p) -> d t p", p=128),
                )
                return kts

            def emit_load_v(s):
                Kv = Ks[s]
                vn = iop.tile([128, KM, 128], BF16, tag="vn")
                nc.sync.dma_start(
                    out=vn[:, :Kv, :],
                    in_=v[s, : Kv * 128, :].rearrange("(t p) d -> p t d", p=128),
                )
                return vn

            def emit_load_q2(s):
                # q in two halves so the first 2 blocks land early
                qs = qp.tile([128, QB, QBW], BF16, tag="qs")
                for h in range(2):
                    nc.sync.dma_start(
                        out=qs[:, 2 * h : 2 * h + 2, :],
                        in_=qT[s, :, h * L // 2 : (h + 1) * L // 2].rearrange(
                            "d (b w) -> d b w", b=2
                        ),
                    )
                return qs

            # ---- rest of the load schedule on the sync queue, in order of
            # first use: k tiles 1.. (2nd exp unit), q block1 (3rd unit),
            # v tiles 1.. (2nd unit's PV). (Measured dead end: splitting
            # these into finer per-use triggers — the extra ~600ns triggers
            # delay everything downstream more than early data helps.)
            if KvA > 1:
                nc.sync.dma_start(
                    out=ktsA[:, 1:KvA, :],
                    in_=kT[sA, :, 128 : KvA * 128].rearrange(
                        "d (t p) -> d t p", p=128
                    ),
                )
            nc.sync.dma_start(out=qsA[:, 1, :], in_=qT[sA, :, QBW : 2 * QBW])
            if KvA > 1:
                nc.sync.dma_start(
                    out=vnA[:, 1:KvA, :],
                    in_=v[sA, 128 : KvA * 128, :].rearrange(
                        "(t p) d -> p t d", p=128
                    ),
                )
            # slot B's k/v next (needed ~6us in), then A's q tail, then B's q
            sB = order[1]
            ktsB = emit_load_k(sB)
            vnB = emit_load_v(sB)
            nc.sync.dma_start(
                out=qsA[:, 2:4, :],
                in_=qT[sA, :, 2 * QBW :].rearrange("d (b w) -> d b w", b=2),
            )
            qsB = emit_load_q2(sB)

            preload = {sA: (ktsA, vnA, qsA), sB: (ktsB, vnB, qsB)}

            # ---- build the GLOBAL unit list: (slot-idx, qb, j0, n, flags).
            # Flattening across slots lets the S-matmul lookahead span slot
            # boundaries, so the PE never drains at a slot switch.
            # unit = (idx, qb, j0, n, is_first, is_last, c0, cw): key tiles
            # [j0, j0+n) x q columns [c0, c0+cw) of block qb of slot idx.
            def slot_units(idx):
                s = order[idx]
                Kv = Ks[s]
                units = []
                for qb in range(QB):
                    if idx == 0 and qb == 0:
                        # fill phase: (1, rest) tile groups so the first exp
                        # fires as soon as k tile 0 + q block 0 land
                        sub = [(0, 1, 0, QBW, True)]
                        if Kv >= 3:
                            sub.append((1, 2, 0, QBW, False))
                        elif Kv == 2:
                            sub.append((1, 1, 0, QBW, False))
                        j = sum(x[1] for x in sub)
                        while j < Kv:
                            n = min(3, Kv - j)
                            sub.append((j, n, 0, QBW, False))
                            j += n
                        n_u = len(sub)
                        for gi, (j0, n, c0, cw, lf) in enumerate(sub):
                            units.append(
                                (idx, qb, j0, n, gi == 0, gi == n_u - 1, c0, cw, lf)
                            )
                        continue
                    groups = []
                    j = 0
                    if idx == SLOTS - 1 and qb == QB - 1 and Kv == 3:
                        # last block of the last slot ends with a 1-tile
                        # group so the final exp->PV->evict->store chain is
                        # as short as possible
                        groups = [(0, 2), (2, 1)]
                    else:
                        while j < Kv:
                            n = min(3, Kv - j)
                            groups.append((j, n))
                            j += n
                    for gi, (j0, n) in enumerate(groups):
                        units.append(
                            (
                                idx,
                                qb,
                                j0,
                                n,
                                gi == 0,
                                gi == len(groups) - 1,
                                0,
                                QBW,
                                gi == 0,
                            )
                        )
                return units

            gunits = []
            slot_first_u = []
            for idx in range(SLOTS):
                slot_first_u.append(len(gunits))
                gunits += slot_units(idx)
            U = len(gunits)
            sts = [None] * U
            state = {}  # per-slot tiles, filled lazily

            def slot_state(idx):
                if idx not in state:
                    s = order[idx]
                    kts, vn, qs = preload.pop(s)
                    state[idx] = {
                        "kts": kts,
                        "vn": vn,
                        "qs": qs,
                        "laccs": lp.tile(
                            [128, QB, QBW], BF16, tag="laccs", name="laccs"
                        ),
                        "o_sb": op_.tile(
                            [128, QB, QBW], BF16, tag="o_sb", name="o_sb"
                        ),
                        "oT_pss": [None] * QB,
                    }
                return state[idx]

            def emit_s(u):
                idx, qb, j0, n, _, _, c0, cw, _ = gunits[u]
                st_ = slot_state(idx)
                # prefetch the next slot's tensors when a slot's S-phase opens
                if idx + 1 < SLOTS and u == slot_first_u[idx]:
                    nxt = order[idx + 1]
                    if nxt in preload or idx + 1 in state:
                        pass
                    else:
                        preload[nxt] = (
                            emit_load_k(nxt),
                            emit_load_v(nxt),
                            emit_load_q2(nxt),
                        )
                st = psst.tile([128, 3, 512], F32, tag="st")
                for jj in range(n):
                    if cw == QBW:
                        nc.tensor.matmul(
                            st[:, jj, :],
                            st_["kts"][:, j0 + jj, :],
                            st_["qs"][:, qb, :],
                            start=True,
                            stop=True,
                        )
                    else:
                        nc.tensor.matmul(
                            st[:, jj, :cw],
                            st_["kts"][:, j0 + jj, :],
                            st_["qs"][:, qb, c0 : c0 + cw],
                            start=True,
                            stop=True,
                        )
                sts[u] = st

            def emit_consume(u):
                idx, qb, j0, n, is_first, is_last, c0, cw, lacc_first = gunits[u]
                s = order[idx]
                Kv = Ks[s]
                last_slot = idx == SLOTS - 1
                st_ = state[idx]
                laccs, o_sb = st_["laccs"], st_["o_sb"]
                st = sts[u]
                full = cw == QBW
                lacc = laccs[:, qb, :] if full else laccs[:, qb, c0 : c0 + cw]
                if is_first:
                    st_["oT_pss"][qb] = psot.tile(
                        [128, QBW], F32, tag="oT", name="oT_ps"
                    )
                oT_ps = st_["oT_pss"][qb]
                pT = workp.tile([128, 3, QBW], BF16, tag="pT")
                if full:
                    nc.scalar.activation(
                        pT[:, :n, :], st[:, :n, :], EXPF, scale=INV_SQRT_D
                    )
                else:
                    nc.scalar.activation(
                        pT[:, :n, :cw], st[:, :n, :cw], EXPF, scale=INV_SQRT_D
                    )
                for jj in range(n):
                    jf = j0 + jj
                    nc.tensor.matmul(
                        oT_ps if full else oT_ps[:, c0 : c0 + cw],
                        st_["vn"][:, jf, :],
                        pT[:, jj, :] if full else pT[:, jj, :cw],
                        start=(jf == 0),
                        stop=(jf == Kv - 1),
                    )
                # denominator accumulation on the DVE (bf16 2x rate)
                base = 0
                if lacc_first:
                    if n >= 2:
                        nc.vector.tensor_add(
                            lacc,
                            pT[:, 0, :] if full else pT[:, 0, :cw],
                            pT[:, 1, :] if full else pT[:, 1, :cw],
                        )
                        base = 2
                    else:
                        nc.vector.tensor_copy(
                            lacc, pT[:, 0, :] if full else pT[:, 0, :cw]
                        )
                        base = 1
                for jj in range(base, n):
                    nc.vector.tensor_add(
                        lacc, lacc, pT[:, jj, :] if full else pT[:, jj, :cw]
                    )
                if is_last:
                    # evict O^T (cast to bf16) on the DVE
                    nc.vector.tensor_copy(o_sb[:, qb, :], oT_ps)
                    if last_slot:
                        # per-block stores; oT rides sync, the lacc blocks
                        # of the second half ride the (idle by now) scalar
                        # queue so the end-of-kernel drain uses two rings
                        # in parallel, ~128KB each
                        nc.sync.dma_start(
                            out=oT[s, :, qb * QBW : (qb + 1) * QBW],
                            in_=o_sb[:, qb, :],
                        )
                        if qb == 1:
                            nc.sync.dma_start(
                                out=lout[s, :, 0:2, :],
                                in_=laccs[:, 0:2, :],
                            )
                        elif qb >= 2:
                            nc.scalar.dma_start(
                                out=lout[s, :, qb : qb + 1, :],
                                in_=laccs[:, qb : qb + 1, :],
                            )
                    else:
                        # mid-kernel stores ride SWDGE (gpsimd), keeping
                        # both HWDGE queues free for loads
                        if qb % 2 == 1:
                            nc.gpsimd.dma_start(
                                out=oT[s, :, (qb - 1) * QBW : (qb + 1) * QBW]
                                .rearrange("d (b w) -> d b w", b=2),
                                in_=o_sb[:, qb - 1 : qb + 1, :],
                            )
                        if qb == QB - 1:
                            nc.gpsimd.dma_start(
                                out=lout[s, :, :, :], in_=laccs[:, :, :]
                            )

            for u in range(min(2, U)):
                emit_s(u)
            for u in range(2, U):
                emit_s(u)
                emit_consume(u - 2)
            for u in range(max(0, U - 2), U):
                emit_consume(u)
    nc.compile()
    return nc


def _get_program(K0: int, K1: int):
    key = (K0, K1)
    if key not in _cache:
        _cache[key] = _build(K0, K1)
    return _cache[key]


def _run(q, k, v, valid_lens, trace=False):
    import ml_dtypes

    BF = ml_dtypes.bfloat16
    q = np.asarray(q, dtype=np.float32)
    k = np.asarray(k, dtype=np.float32)
    v = np.asarray(v, dtype=np.float32)
    vl = np.asarray(valid_lens).astype(np.int64)
    K0 = int(max(1, -(-vl[0] // 128)))
    K1 = int(max(1, -(-vl[1] // 128)))
    KM = max(K0, K1)
    nc = _get_program(K0, K1)

    Ks = [K0, K0, K1, K1]
    bs = [0, 0, 1, 1]
    nmask = [Ks[i] * 128 - int(vl[bs[i]]) for i in range(SLOTS)]

    # zero masked key positions once for the whole tensor (shared across cores)
    kz = k[:, :, : KM * 128, :].copy()
    vz = v[:, :, : KM * 128, :].astype(BF)
    for b in range(B):
        kz[b, :, vl[b] :, :] = 0.0
        vz[b, :, vl[b] :, :] = 0.0
    # [B, H, D, KM*128] transposed keys / queries in bf16
    kzT = np.ascontiguousarray(kz.transpose(0, 1, 3, 2)).astype(BF)
    qT_full = np.ascontiguousarray(q.transpose(0, 1, 3, 2)).astype(BF)

    in_maps = []
    for c in range(NCORES):
        h0, h1 = 2 * c, 2 * c + 1
        qts = np.ascontiguousarray(
            np.stack([qT_full[0, h0], qT_full[0, h1], qT_full[1, h0], qT_full[1, h1]])
        )
        kts = np.ascontiguousarray(
            np.stack([kzT[0, h0], kzT[0, h1], kzT[1, h0], kzT[1, h1]])
        )
        vs = np.ascontiguousarray(
            np.stack([vz[0, h0], vz[0, h1], vz[1, h0], vz[1, h1]])
        )
        in_maps.append({"qT": qts, "kT": kts, "v": vs})

    try:
        res = run_bass_kernel_spmd(
            nc, in_maps, core_ids=list(range(NCORES)), trace=trace
        )
    except Exception:
        # transient device wedges (NRT_EXEC_UNIT_UNRECOVERABLE) have been
        # observed to clear on retry
        res = run_bass_kernel_spmd(
            nc, in_maps, core_ids=list(range(NCORES)), trace=trace
        )

    outp = np.empty((B, H, L, D), dtype=np.float32)
    for c in range(NCORES):
        oT_dev = res.results[c]["oT"]
        l_dev = res.results[c]["lout"]
        h0, h1 = 2 * c, 2 * c + 1
        for i, (b, h) in enumerate([(0, h0), (0, h1), (1, h0), (1, h1)]):
            l = l_dev[i].astype(np.float32).sum(axis=0).reshape(L) - nmask[i]
            outp[b, h] = oT_dev[i].astype(np.float32).T / l[:, None]
    return outp, res


def kernel(q, k, v, valid_lens):
    outp, _ = _run(q, k, v, valid_lens, trace=False)
    return outp
